# revision 23
# baseline (speedup 1.0000x reference)
"""Additive-attention pooling kernel for 8 TRN2 NeuronCores.

reference:
    h     = tanh(x @ (W1+W2) + (b1+b2))      x: [B, T, D]
    score = h @ V + V_b                      [B, T, 1]
    attn  = softmax(score, axis=T)
    out   = sum_t attn * x                   [B, D]

Sharding: data-parallel over batch; each of the 8 cores gets B/8 = 8
examples (8 MB of fp32), weights replicated. No collectives.

Layout: token t = c*2048 + p*16 + i lands on partition p, tile i of
example c (token order within an example is permuted vs the reference;
softmax pooling is permutation-invariant).  Each partition reads
contiguous 2-8 KB runs from HBM.

v2 data path (vs the v1 SWDGE-cast design): x streams as plain f32
over the HARDWARE DGE (SP + ACT queues), which starts at ~4 us and
runs at HBM line rate, instead of the software-DGE cast path that was
desc-gen paced (~288 GB/s) and started at ~9 us behind the Q7
preamble.  f32 -> bf16 casts are split across DVE (c0, c1, part of
c7) and the otherwise-idle GPSIMD/Pool engine (c2..c6, rest of c7).
Identities are built on DVE so Q7 never blocks anything.

Per-core compute per 4-tile cluster: PE transposes x (xT) via the
identity; PE h-matmul with stationary Wc -> psum; ACT tanh(+bias) ->
hT sbuf; PE score matmul (stationary hT tile, 1-col rhs v) -> psum
column; ACT exp per example -> masked e-block column with accum_out
denominator; PE context matmul (stationary x tile, 8-col e-block rhs)
accumulating ctx^T [d, 8] across all 128 tiles in one psum bank.
PE emission is software-pipelined: transposes+h-matmuls of example c
run before scores of c (hiding the tanh latency), and ctx of c-1 is
interleaved into c's slot.  First and last examples run at 4-tile
granularity for ramp/tail latency; the middle at 8-tile.

V_b is omitted: softmax(score + c) == softmax(score) exactly.
Softmax runs without max-subtraction: |score| <= sum|V_u| ~ 9.
"""

import os

import numpy as np

B, T, D, U = 64, 2048, 128, 128
N_CORES = 8
B_LOC = B // N_CORES          # 8 examples per core
N_TILE_EX = T // 128          # 16 token-tiles per example
N_TILES = B_LOC * N_TILE_EX   # 128 tiles per core

_nc = None
LAST_RESULT = None


def _build_nc():
    import concourse.bass as bass  # noqa: F401
    import concourse.mybir as mybir
    import concourse.tile as tile
    from concourse import bacc

    f32 = mybir.dt.float32
    bf16 = mybir.dt.bfloat16
    ACT = mybir.ActivationFunctionType
    ALU = mybir.AluOpType

    nc = bacc.Bacc("TRN2", target_bir_lowering=False, debug=False,
                   num_devices=N_CORES)

    x_d = nc.declare_dram_parameter("encoder_outputs", [B_LOC * T, D], f32,
                                    isOutput=False)
    w1_d = nc.declare_dram_parameter("W1_w", [D, U], f32, isOutput=False)
    b1_d = nc.declare_dram_parameter("W1_b", [U, 1], f32, isOutput=False)
    w2_d = nc.declare_dram_parameter("W2_w", [D, U], f32, isOutput=False)
    b2_d = nc.declare_dram_parameter("W2_b", [U, 1], f32, isOutput=False)
    v_d = nc.declare_dram_parameter("V_w", [U, 1], f32, isOutput=False)
    out_d = nc.declare_dram_parameter("out", [B_LOC, D], f32, isOutput=True)

    # token = c*T + p*16 + i  ->  [c][p][i][d]; per-(c,p) HBM runs are 8 KB
    x_r = x_d.ap().rearrange("(c p i) d -> c p i d", c=B_LOC, p=128,
                             i=N_TILE_EX)

    def make_ident(eng, dst):
        eng.memset(dst, 0.0)
        eng.affine_select(out=dst, in_=dst, compare_op=ALU.not_equal,
                          fill=1.0, base=0, pattern=[[-1, dst.shape[1]]],
                          channel_multiplier=1)

    with tile.TileContext(nc) as tc:
        with (
            tc.tile_pool(name="consts", bufs=1) as consts,
            tc.tile_pool(name="big", bufs=1) as big,
            tc.tile_pool(name="ps_xt", bufs=2, space="PSUM") as ps_xt_pool,
            tc.tile_pool(name="ps_h", bufs=2, space="PSUM") as ps_h_pool,
            tc.tile_pool(name="ps_sc", bufs=1, space="PSUM") as ps_sc_pool,
            tc.tile_pool(name="ps_cx", bufs=1, space="PSUM") as ps_cx_pool,
        ):
            # ---- persistent buffers ----
            xf = big.tile([128, 2 * N_TILE_EX, 128], f32)   # c1/c2 staging
            x_bf = big.tile([128, N_TILES * 128], bf16)     # 4 MB  [t, d]
            xT = big.tile([128, N_TILES * 128], bf16)       # 4 MB  [d, t]
            ht = big.tile([128, N_TILES * 128], bf16)       # 4 MB tanh(h)^T
            eb = big.tile([128, N_TILES * 8], bf16)         # masked e-blocks
            e_all = big.tile([128, 8], f32)
            e_7a = big.tile([128, 1], f32)
            e_7b = big.tile([128, 1], f32)
            cxT_sb = big.tile([128, 8], f32)
            out_sb = big.tile([B_LOC, 128], f32)
            den_r = big.tile([B_LOC, 1], f32)

            x_bf_r = x_bf.rearrange("p (j d) -> p j d", d=128)
            eb_r = eb.rearrange("p (j e) -> p j e", e=8)

            ident = consts.tile([128, 128], bf16)
            ident_f = consts.tile([128, 128], f32)
            w1_sb = consts.tile([128, 128], f32)
            w2_sb = consts.tile([128, 128], f32)
            wc_bf = consts.tile([128, 128], bf16)   # [d, u] stationary
            b1_sb = consts.tile([128, 1], f32)
            b2_sb = consts.tile([128, 1], f32)
            b_sum = consts.tile([128, 1], f32)      # per-partition (=u) bias
            vf_sb = consts.tile([128, 1], f32)
            v_bf = consts.tile([128, 1], bf16)
            ones_col = consts.tile([128, 1], f32)
            act_warm = consts.tile([128, 1], f32)

            ps_sc = ps_sc_pool.tile([128, 128], f32)   # score col per tile
            ps_cx = ps_cx_pool.tile([128, 512], f32)

            # ================= DMA issue =================
            # Every queue pays a ~6.5us framework preamble, so nothing moves
            # before ~7.5us.  ALL of x rides ONE SWDGE casting stream on Q7
            # in consumption order (f32 HBM -> bf16 SBUF inside the DMA
            # engines; a competing HWDGE x stream just steals engine slots
            # from the head of this one — measured in v3).  The stream leads
            # with fine c0 chunks; the bf16 identity builds while c0's first
            # bytes are in flight.
            x_chunks = [(0, 2), (2, 2), None,        # c0: 2+2, then ident
                        (4, 4), (8, 8)]              # c0 rest
            x_chunks += [(16 * c, 16) for c in range(3, 7)]
            x_chunks += [(112 + 4 * q, 4) for q in range(4)]  # c7 quarters
            for item in x_chunks:
                if item is None:
                    make_ident(nc.gpsimd, ident)
                    continue
                lo, w = item
                src_c, src_lo = lo // 16, lo % 16
                nc.gpsimd.dma_start(
                    out=x_bf_r[:, lo:lo + w],
                    in_=x_r[src_c][:, src_lo:src_lo + w])
            make_ident(nc.gpsimd, ident_f)   # needed only at the tail

            # weights + c1/c2 (f32, DVE-cast later) on the SP HWDGE: takes
            # ~5.6us of cast-tax volume off the SWDGE stream's tail
            nc.sync.dma_start(out=w1_sb, in_=w1_d.ap())
            nc.sync.dma_start(out=w2_sb, in_=w2_d.ap())
            for h in range(4):
                src_c = 1 + h // 2
                lo = 8 * (h % 2)
                nc.sync.dma_start(out=xf[:, 16 * (src_c - 1) + lo:
                                         16 * (src_c - 1) + lo + 8],
                                  in_=x_r[src_c][:, lo:lo + 8])
            # bias/v configs on the ACT queue (behind the hoisted
            # activation-table load; land in time for first tanh/score)
            nc.scalar.dma_start(out=b1_sb, in_=b1_d.ap())
            nc.scalar.dma_start(out=b2_sb, in_=b2_d.ap())
            nc.scalar.dma_start(out=vf_sb, in_=v_d.ap())

            # scratch for the PE HAM warm-up (memset first so the dummy
            # matmuls never read uninitialized SBUF)
            scratch = consts.tile([128, 512], bf16)
            nc.vector.memset(scratch, 0.5)
            nc.vector.memset(ones_col, 1.0)
            # ACT: warm the exp/tanh table during the DMA ramp
            nc.scalar.activation(act_warm, ones_col, ACT.Exp)

            # PE warm-up: real (non-transpose) matmuls in the otherwise-dead
            # window before c0 lands, to pull the HAM full-clock flip
            # earlier.  They chain WAW on one psum tile; done before the
            # first real transpose needs the pool.
            ph_warm = ps_h_pool.tile([128, 1024], f32, tag="ph")
            for _ in range(8):
                nc.tensor.matmul(ph_warm[:, 0:512], lhsT=scratch[:, 0:128],
                                 rhs=scratch)

            nc.vector.tensor_add(wc_bf, w1_sb, w2_sb)
            nc.vector.tensor_add(b_sum, b1_sb, b2_sb)
            nc.vector.tensor_copy(v_bf, vf_sb)
            nc.vector.memset(eb, 0.0)

            # ================= main pipeline =================
            # Per example c: PE does transposes + h-matmuls for both halves,
            # then ctx(c-1), then scores(c).  ACT does exp(c-1) then tanh(c).
            def transposes(j0, w):
                """One w-tile cluster starting at tile j0 -> xT via psum.
                w=8 amortizes the DVE psum-drain copy; w=4 for ramp/tail."""
                pxt = ps_xt_pool.tile([128, 128 * w], bf16, tag="pxt")
                for r in range(w):
                    nc.tensor.transpose(
                        pxt[:, 128 * r:128 * (r + 1)],
                        x_bf_r[:, j0 + r], ident)
                s = 128 * j0
                nc.vector.tensor_copy(xT[:, s:s + 128 * w], pxt)

            def ctx_mms(c):
                for i in range(N_TILE_EX):
                    j = 16 * c + i
                    nc.tensor.matmul(ps_cx[:, 0:8], lhsT=x_bf_r[:, j],
                                     rhs=eb_r[:, j],
                                     start=(j == 0), stop=(j == N_TILES - 1))

            def scores(j0, n):
                for i in range(n):
                    j = j0 + i
                    nc.tensor.matmul(ps_sc[:, j:j + 1],
                                     lhsT=ht[:, 128 * j:128 * (j + 1)],
                                     rhs=v_bf)

            def exp_c(c):
                nc.scalar.activation(eb_r[:, 16 * c:16 * c + 16, c],
                                     ps_sc[:, 16 * c:16 * c + 16],
                                     ACT.Exp, accum_out=e_all[:, c:c + 1])

            def h_mm(ph, base, lo, hi):
                nc.tensor.matmul(ph[:, lo:hi], lhsT=wc_bf,
                                 rhs=xT[:, base + lo:base + hi])

            def cast_half(h):
                # DVE f32->bf16 for c1/c2 half h (tiles 16+8h .. 24+8h)
                nc.vector.tensor_copy(x_bf_r[:, 16 + 8 * h:24 + 8 * h],
                                      xf[:, 8 * h:8 * (h + 1)])

            for c in range(B_LOC - 1):
                if c >= 1:
                    # ACT queue: previous example's exp runs while PE does
                    # this example's transposes/h-matmuls
                    exp_c(c - 1)
                for g in range(2):
                    base = 2048 * c + 1024 * g
                    j0 = 16 * c + 8 * g
                    ph = ps_h_pool.tile([128, 1024], f32, tag="ph")
                    if c == 0:   # 4-tile chains for the ramp
                        transposes(j0, 4)
                        h_mm(ph, base, 0, 512)
                        nc.scalar.activation(ht[:, base:base + 512],
                                             ph[:, 0:512], ACT.Tanh,
                                             bias=b_sum)
                        cast_half(g)          # c1 half g lands about now
                        transposes(j0 + 4, 4)
                        h_mm(ph, base, 512, 1024)
                        nc.scalar.activation(ht[:, base + 512:base + 1024],
                                             ph[:, 512:1024], ACT.Tanh,
                                             bias=b_sum)
                    else:
                        if c == 1:
                            cast_half(2 + g)  # c2 halves
                        transposes(j0, 8)
                        h_mm(ph, base, 0, 512)
                        h_mm(ph, base, 512, 1024)
                        nc.scalar.activation(ht[:, base:base + 1024], ph,
                                             ACT.Tanh, bias=b_sum)
                # previous example's ctx rides between c's h and scores
                if c >= 1:
                    ctx_mms(c - 1)
                scores(16 * c, 8)
                scores(16 * c + 8, 8)

            # ---- last example: fully pipelined per-quarter tail ----
            c = B_LOC - 1
            e7q = [e_7a, e_7b,
                   big.tile([128, 1], f32, name="e_7c"),
                   big.tile([128, 1], f32, name="e_7d")]
            phs = []

            def c7_quarter(q):
                j0 = 112 + 4 * q
                if q % 2 == 0:
                    phs.append(ps_h_pool.tile([128, 1024], f32, tag="ph",
                                              name="ph7"))
                ph = phs[-1]
                lo = 512 * (q % 2)
                transposes(j0, 4)
                nc.tensor.matmul(ph[:, lo:lo + 512], lhsT=wc_bf,
                                 rhs=xT[:, 128 * j0:128 * j0 + 512])
                nc.scalar.activation(ht[:, 128 * j0:128 * j0 + 512],
                                     ph[:, lo:lo + 512], ACT.Tanh,
                                     bias=b_sum)

            def sc7(q):
                scores(112 + 4 * q, 4)

            def exp7(q):
                nc.scalar.activation(
                    eb_r[:, 112 + 4 * q:116 + 4 * q, c],
                    ps_sc[:, 112 + 4 * q:116 + 4 * q],
                    ACT.Exp, accum_out=e7q[q])

            def ctx7(q):
                for i in range(4):
                    j = 112 + 4 * q + i
                    nc.tensor.matmul(ps_cx[:, 0:8], lhsT=x_bf_r[:, j],
                                     rhs=eb_r[:, j],
                                     start=(j == 0), stop=(j == N_TILES - 1))

            exp_c(c - 1)
            c7_quarter(0)
            c7_quarter(1)
            ctx_mms(c - 1)
            sc7(0); exp7(0)
            c7_quarter(2)
            sc7(1); exp7(1)
            ctx7(0)
            c7_quarter(3)
            sc7(2); exp7(2)
            ctx7(1)
            sc7(3); exp7(3)
            ctx7(2)
            ctx7(3)

            # ---- denominator + final transpose/scale ----
            s01 = big.tile([128, 1], f32)
            s23 = big.tile([128, 1], f32)
            nc.vector.tensor_add(s01, e7q[0], e7q[1])
            nc.vector.tensor_add(s23, e7q[2], e7q[3])
            nc.vector.tensor_add(e_all[:, c:c + 1], s01, s23)
            nc.tensor.matmul(ps_cx[0:8, 448:449], lhsT=e_all, rhs=ones_col)

            nc.vector.tensor_copy(cxT_sb, ps_cx[:, 0:8])
            nc.tensor.transpose(ps_cx[0:8, 320:448], cxT_sb, ident_f)

            nc.vector.reciprocal(den_r, ps_cx[0:8, 448:449])
            nc.vector.tensor_scalar_mul(out_sb, ps_cx[0:8, 320:448], den_r)
            nc.sync.dma_start(out=out_d.ap(), in_=out_sb)

    nc.compile()
    return nc


def get_nc():
    global _nc
    if _nc is None:
        _nc = _build_nc()
    return _nc


def kernel(encoder_outputs, W1_w, W1_b, W2_w, W2_b, V_w, V_b):
    global LAST_RESULT
    from concourse.bass_utils import run_bass_kernel_spmd

    nc = get_nc()

    enc = np.ascontiguousarray(np.asarray(encoder_outputs, dtype=np.float32))
    rep = {
        "W1_w": np.ascontiguousarray(np.asarray(W1_w, np.float32)),
        "W1_b": np.ascontiguousarray(np.asarray(W1_b, np.float32).reshape(U, 1)),
        "W2_w": np.ascontiguousarray(np.asarray(W2_w, np.float32)),
        "W2_b": np.ascontiguousarray(np.asarray(W2_b, np.float32).reshape(U, 1)),
        "V_w": np.ascontiguousarray(np.asarray(V_w, np.float32).reshape(U, 1)),
    }
    in_maps = []
    for c in range(N_CORES):
        shard = enc[c * B_LOC:(c + 1) * B_LOC].reshape(B_LOC * T, D)
        in_maps.append({"encoder_outputs": np.ascontiguousarray(shard), **rep})

    trace = bool(int(os.environ.get("KERNEL_TRACE", "0")))
    LAST_RESULT = run_bass_kernel_spmd(
        nc, in_maps, core_ids=list(range(N_CORES)), trace=trace)
    out = np.concatenate(
        [LAST_RESULT.results[c]["out"] for c in range(N_CORES)], axis=0)
    return np.ascontiguousarray(out, dtype=np.float32)


# revision 31
# speedup vs baseline: 1.0829x; 1.0829x over previous
"""Additive-attention pooling kernel for 8 TRN2 NeuronCores.

reference:
    h     = tanh(x @ (W1+W2) + (b1+b2))      x: [B, T, D]
    score = h @ V + V_b                      [B, T, 1]
    attn  = softmax(score, axis=T)
    out   = sum_t attn * x                   [B, D]

Sharding: data-parallel over batch; each of the 8 cores gets B/8 = 8
examples (8 MB of fp32), weights replicated. No collectives.

Layout: token t = c*2048 + p*16 + i lands on partition p, tile i of
example c (token order within an example is permuted vs the reference;
softmax pooling is permutation-invariant).  Each partition reads
contiguous 2-8 KB runs from HBM.

v2 data path (vs the v1 SWDGE-cast design): x streams as plain f32
over the HARDWARE DGE (SP + ACT queues), which starts at ~4 us and
runs at HBM line rate, instead of the software-DGE cast path that was
desc-gen paced (~288 GB/s) and started at ~9 us behind the Q7
preamble.  f32 -> bf16 casts are split across DVE (c0, c1, part of
c7) and the otherwise-idle GPSIMD/Pool engine (c2..c6, rest of c7).
Identities are built on DVE so Q7 never blocks anything.

Per-core compute per 4-tile cluster: PE transposes x (xT) via the
identity; PE h-matmul with stationary Wc -> psum; ACT tanh(+bias) ->
hT sbuf; PE score matmul (stationary hT tile, 1-col rhs v) -> psum
column; ACT exp per example -> masked e-block column with accum_out
denominator; PE context matmul (stationary x tile, 8-col e-block rhs)
accumulating ctx^T [d, 8] across all 128 tiles in one psum bank.
PE emission is software-pipelined: transposes+h-matmuls of example c
run before scores of c (hiding the tanh latency), and ctx of c-1 is
interleaved into c's slot.  First and last examples run at 4-tile
granularity for ramp/tail latency; the middle at 8-tile.

V_b is omitted: softmax(score + c) == softmax(score) exactly.
Softmax runs without max-subtraction: |score| <= sum|V_u| ~ 9.
"""

import os

import numpy as np

B, T, D, U = 64, 2048, 128, 128
N_CORES = 8
B_LOC = B // N_CORES          # 8 examples per core
N_TILE_EX = T // 128          # 16 token-tiles per example
N_TILES = B_LOC * N_TILE_EX   # 128 tiles per core

_nc = None
LAST_RESULT = None


def _build_nc():
    import concourse.bass as bass  # noqa: F401
    import concourse.mybir as mybir
    import concourse.tile as tile
    from concourse import bacc

    f32 = mybir.dt.float32
    bf16 = mybir.dt.bfloat16
    ACT = mybir.ActivationFunctionType
    ALU = mybir.AluOpType

    nc = bacc.Bacc("TRN2", target_bir_lowering=False, debug=False,
                   num_devices=N_CORES)

    x_d = nc.declare_dram_parameter("encoder_outputs", [B_LOC * T, D], f32,
                                    isOutput=False)
    w1_d = nc.declare_dram_parameter("W1_w", [D, U], f32, isOutput=False)
    b1_d = nc.declare_dram_parameter("W1_b", [U, 1], f32, isOutput=False)
    w2_d = nc.declare_dram_parameter("W2_w", [D, U], f32, isOutput=False)
    b2_d = nc.declare_dram_parameter("W2_b", [U, 1], f32, isOutput=False)
    v_d = nc.declare_dram_parameter("V_w", [U, 1], f32, isOutput=False)
    out_d = nc.declare_dram_parameter("out", [B_LOC, D], f32, isOutput=True)

    # token = c*T + p*16 + i  ->  [c][p][i][d]; per-(c,p) HBM runs are 8 KB
    x_r = x_d.ap().rearrange("(c p i) d -> c p i d", c=B_LOC, p=128,
                             i=N_TILE_EX)

    def make_ident(eng, dst):
        eng.memset(dst, 0.0)
        eng.affine_select(out=dst, in_=dst, compare_op=ALU.not_equal,
                          fill=1.0, base=0, pattern=[[-1, dst.shape[1]]],
                          channel_multiplier=1)

    with tile.TileContext(nc) as tc:
        with (
            tc.tile_pool(name="consts", bufs=1) as consts,
            tc.tile_pool(name="big", bufs=1) as big,
            tc.tile_pool(name="ps_xt", bufs=2, space="PSUM") as ps_xt_pool,
            tc.tile_pool(name="ps_h", bufs=2, space="PSUM") as ps_h_pool,
            tc.tile_pool(name="ps_sc", bufs=1, space="PSUM") as ps_sc_pool,
            tc.tile_pool(name="ps_cx", bufs=1, space="PSUM") as ps_cx_pool,
        ):
            # ---- persistent buffers ----
            x_bf = big.tile([128, N_TILES * 128], bf16)     # 4 MB  [t, d]
            xT = big.tile([128, N_TILES * 128], bf16)       # 4 MB  [d, t]
            ht = big.tile([128, N_TILES * 128], bf16)       # 4 MB tanh(h)^T
            eb = big.tile([128, N_TILES * 8], bf16)         # masked e-blocks
            e_all = big.tile([128, 8], f32)
            e_7a = big.tile([128, 1], f32)
            e_7b = big.tile([128, 1], f32)
            cxT_sb = big.tile([128, 8], f32)
            out_sb = big.tile([B_LOC, 128], f32)
            den_r = big.tile([B_LOC, 1], f32)

            x_bf_r = x_bf.rearrange("p (j d) -> p j d", d=128)
            eb_r = eb.rearrange("p (j e) -> p j e", e=8)

            ident = consts.tile([128, 128], bf16)
            ident_f = consts.tile([128, 128], f32)
            w1_sb = consts.tile([128, 128], f32)
            w2_sb = consts.tile([128, 128], f32)
            wc_bf = consts.tile([128, 128], bf16)   # [d, u] stationary
            b1_sb = consts.tile([128, 1], f32)
            b2_sb = consts.tile([128, 1], f32)
            b_sum = consts.tile([128, 1], f32)      # per-partition (=u) bias
            vf_sb = consts.tile([128, 1], f32)
            v_bf = consts.tile([128, 1], bf16)
            ones_col = consts.tile([128, 1], f32)
            act_warm = consts.tile([128, 1], f32)

            ps_sc = ps_sc_pool.tile([128, 128], f32)   # score col per tile
            ps_cx = ps_cx_pool.tile([128, 512], f32)

            # ================= DMA issue =================
            # Every queue pays a ~6.5us framework preamble, so nothing moves
            # before ~7.5us.  ALL of x rides ONE SWDGE casting stream on Q7
            # in consumption order (f32 HBM -> bf16 SBUF inside the DMA
            # engines; a competing HWDGE x stream just steals engine slots
            # from the head of this one — measured in v3).  The stream leads
            # with fine c0 chunks; the bf16 identity builds while c0's first
            # bytes are in flight.
            x_chunks = [(0, 2), (2, 2), None,        # c0: 2+2, then ident
                        (4, 4), (8, 8),              # c0 rest
                        (16, 8), (24, 8)]            # c1 halves
            x_chunks += [(16 * c, 16) for c in range(2, 7)]
            # last example: 4+4+4+2+2 (ever-finer tail)
            x_chunks += [(112, 4), (116, 4), (120, 4), (124, 2), (126, 2)]
            for item in x_chunks:
                if item is None:
                    make_ident(nc.gpsimd, ident)
                    continue
                lo, w = item
                src_c, src_lo = lo // 16, lo % 16
                nc.gpsimd.dma_start(
                    out=x_bf_r[:, lo:lo + w],
                    in_=x_r[src_c][:, src_lo:src_lo + w])
            make_ident(nc.gpsimd, ident_f)   # needed only at the tail

            # weights on the SP HWDGE (idle engines before the stream arms)
            nc.sync.dma_start(out=w1_sb, in_=w1_d.ap())
            nc.sync.dma_start(out=w2_sb, in_=w2_d.ap())
            # bias/v configs on the ACT queue (behind the hoisted
            # activation-table load; land in time for first tanh/score)
            nc.scalar.dma_start(out=b1_sb, in_=b1_d.ap())
            nc.scalar.dma_start(out=b2_sb, in_=b2_d.ap())
            nc.scalar.dma_start(out=vf_sb, in_=v_d.ap())

            # scratch for the PE HAM warm-up (memset first so the dummy
            # matmuls never read uninitialized SBUF)
            scratch = consts.tile([128, 512], bf16)
            nc.vector.memset(scratch, 0.5)
            nc.vector.memset(ones_col, 1.0)
            # ACT: warm the exp/tanh table during the DMA ramp
            nc.scalar.activation(act_warm, ones_col, ACT.Exp)

            # PE warm-up: real (non-transpose) matmuls in the otherwise-dead
            # window before c0 lands, to pull the HAM full-clock flip
            # earlier.  They chain WAW on one psum tile; done before the
            # first real transpose needs the pool.
            ph_warm = ps_h_pool.tile([128, 1024], f32, tag="ph")
            for _ in range(6):
                nc.tensor.matmul(ph_warm[:, 0:512], lhsT=scratch[:, 0:128],
                                 rhs=scratch)

            nc.vector.tensor_add(wc_bf, w1_sb, w2_sb)
            nc.vector.tensor_add(b_sum, b1_sb, b2_sb)
            nc.vector.tensor_copy(v_bf, vf_sb)
            nc.vector.memset(eb, 0.0)

            # ================= main pipeline =================
            # Per example c: PE does transposes + h-matmuls for both halves,
            # then ctx(c-1), then scores(c).  ACT does exp(c-1) then tanh(c).
            def transposes(j0, w):
                """One w-tile cluster starting at tile j0 -> xT via psum.
                w=8 amortizes the DVE psum-drain copy; w=4 for ramp/tail."""
                pxt = ps_xt_pool.tile([128, 128 * w], bf16, tag="pxt")
                for r in range(w):
                    nc.tensor.transpose(
                        pxt[:, 128 * r:128 * (r + 1)],
                        x_bf_r[:, j0 + r], ident)
                s = 128 * j0
                nc.vector.tensor_copy(xT[:, s:s + 128 * w], pxt)

            def ctx_mms(c):
                for i in range(N_TILE_EX):
                    j = 16 * c + i
                    nc.tensor.matmul(ps_cx[:, 0:8], lhsT=x_bf_r[:, j],
                                     rhs=eb_r[:, j],
                                     start=(j == 0), stop=(j == N_TILES - 1))

            def scores(j0, n):
                for i in range(n):
                    j = j0 + i
                    nc.tensor.matmul(ps_sc[:, j:j + 1],
                                     lhsT=ht[:, 128 * j:128 * (j + 1)],
                                     rhs=v_bf)

            def exp_c(c):
                nc.scalar.activation(eb_r[:, 16 * c:16 * c + 16, c],
                                     ps_sc[:, 16 * c:16 * c + 16],
                                     ACT.Exp, accum_out=e_all[:, c:c + 1])

            def h_mm(ph, base, lo, hi):
                nc.tensor.matmul(ph[:, lo:hi], lhsT=wc_bf,
                                 rhs=xT[:, base + lo:base + hi])

            for c in range(B_LOC - 1):
                if c >= 1:
                    # ACT queue: previous example's exp runs while PE does
                    # this example's transposes/h-matmuls
                    exp_c(c - 1)
                for g in range(2):
                    base = 2048 * c + 1024 * g
                    j0 = 16 * c + 8 * g
                    ph = ps_h_pool.tile([128, 1024], f32, tag="ph")
                    if c == 0:   # 4-tile chains for the ramp
                        transposes(j0, 4)
                        h_mm(ph, base, 0, 512)
                        nc.scalar.activation(ht[:, base:base + 512],
                                             ph[:, 0:512], ACT.Tanh,
                                             bias=b_sum)
                        transposes(j0 + 4, 4)
                        h_mm(ph, base, 512, 1024)
                        nc.scalar.activation(ht[:, base + 512:base + 1024],
                                             ph[:, 512:1024], ACT.Tanh,
                                             bias=b_sum)
                    else:
                        transposes(j0, 8)
                        h_mm(ph, base, 0, 512)
                        h_mm(ph, base, 512, 1024)
                        nc.scalar.activation(ht[:, base:base + 1024], ph,
                                             ACT.Tanh, bias=b_sum)
                # previous example's ctx rides between c's h and scores
                if c >= 1:
                    ctx_mms(c - 1)
                scores(16 * c, 8)
                scores(16 * c + 8, 8)

            # ---- last example: fully pipelined ever-finer tail ----
            # chunks of 4,4,4,2,2 tiles; tanh/scores/exp/ctx per chunk
            c = B_LOC - 1
            CH = [(112, 4), (116, 4), (120, 4), (124, 2), (126, 2)]
            e7q = [e_7a, e_7b,
                   big.tile([128, 1], f32, name="e_7c"),
                   big.tile([128, 1], f32, name="e_7d"),
                   big.tile([128, 1], f32, name="e_7e")]
            # ph tiles: chunks 0-1 share ph0; chunks 2-4 share ph1
            ph7 = [None, None]
            ph_of = [(0, 0), (0, 512), (1, 0), (1, 512), (1, 768)]

            def c7_chunk(q):
                j0, w = CH[q]
                t, lo = ph_of[q]
                if ph7[t] is None or (t, lo) in ((0, 0), (1, 0)):
                    ph7[t] = ps_h_pool.tile([128, 1024], f32, tag="ph",
                                            name="ph7")
                ph = ph7[t]
                transposes(j0, w)
                cols = 128 * w
                nc.tensor.matmul(ph[:, lo:lo + cols], lhsT=wc_bf,
                                 rhs=xT[:, 128 * j0:128 * j0 + cols])
                nc.scalar.activation(ht[:, 128 * j0:128 * j0 + cols],
                                     ph[:, lo:lo + cols], ACT.Tanh,
                                     bias=b_sum)

            def sc7(q):
                scores(CH[q][0], CH[q][1])

            def exp7(q):
                j0, w = CH[q]
                nc.scalar.activation(eb_r[:, j0:j0 + w, c],
                                     ps_sc[:, j0:j0 + w],
                                     ACT.Exp, accum_out=e7q[q])

            def ctx7(q):
                j0, w = CH[q]
                for i in range(w):
                    j = j0 + i
                    nc.tensor.matmul(ps_cx[:, 0:8], lhsT=x_bf_r[:, j],
                                     rhs=eb_r[:, j],
                                     start=(j == 0), stop=(j == N_TILES - 1))

            exp_c(c - 1)
            c7_chunk(0)
            c7_chunk(1)
            ctx_mms(c - 1)
            sc7(0); exp7(0)
            c7_chunk(2)
            sc7(1); exp7(1)
            ctx7(0)
            c7_chunk(3)
            sc7(2); exp7(2)
            ctx7(1)
            c7_chunk(4)
            sc7(3); exp7(3)
            ctx7(2)
            sc7(4); exp7(4)
            ctx7(3)
            ctx7(4)

            # ---- denominator + final transpose/scale ----
            s01 = big.tile([128, 1], f32)
            s23 = big.tile([128, 1], f32)
            s04 = big.tile([128, 1], f32)
            nc.vector.tensor_add(s01, e7q[0], e7q[1])
            nc.vector.tensor_add(s23, e7q[2], e7q[3])
            nc.vector.tensor_add(s04, s01, s23)
            nc.vector.tensor_add(e_all[:, c:c + 1], s04, e7q[4])
            nc.tensor.matmul(ps_cx[0:8, 448:449], lhsT=e_all, rhs=ones_col)

            nc.vector.tensor_copy(cxT_sb, ps_cx[:, 0:8])
            nc.tensor.transpose(ps_cx[0:8, 320:448], cxT_sb, ident_f)

            nc.vector.reciprocal(den_r, ps_cx[0:8, 448:449])
            nc.vector.tensor_scalar_mul(out_sb, ps_cx[0:8, 320:448], den_r)
            nc.sync.dma_start(out=out_d.ap(), in_=out_sb)

    nc.compile()
    return nc


def get_nc():
    global _nc
    if _nc is None:
        _nc = _build_nc()
    return _nc


def kernel(encoder_outputs, W1_w, W1_b, W2_w, W2_b, V_w, V_b):
    global LAST_RESULT
    from concourse.bass_utils import run_bass_kernel_spmd

    nc = get_nc()

    enc = np.ascontiguousarray(np.asarray(encoder_outputs, dtype=np.float32))
    rep = {
        "W1_w": np.ascontiguousarray(np.asarray(W1_w, np.float32)),
        "W1_b": np.ascontiguousarray(np.asarray(W1_b, np.float32).reshape(U, 1)),
        "W2_w": np.ascontiguousarray(np.asarray(W2_w, np.float32)),
        "W2_b": np.ascontiguousarray(np.asarray(W2_b, np.float32).reshape(U, 1)),
        "V_w": np.ascontiguousarray(np.asarray(V_w, np.float32).reshape(U, 1)),
    }
    in_maps = []
    for c in range(N_CORES):
        shard = enc[c * B_LOC:(c + 1) * B_LOC].reshape(B_LOC * T, D)
        in_maps.append({"encoder_outputs": np.ascontiguousarray(shard), **rep})

    trace = bool(int(os.environ.get("KERNEL_TRACE", "0")))
    LAST_RESULT = run_bass_kernel_spmd(
        nc, in_maps, core_ids=list(range(N_CORES)), trace=trace)
    out = np.concatenate(
        [LAST_RESULT.results[c]["out"] for c in range(N_CORES)], axis=0)
    return np.ascontiguousarray(out, dtype=np.float32)


# revision 37
# speedup vs baseline: 1.0847x; 1.0017x over previous
"""Additive-attention pooling kernel for 8 TRN2 NeuronCores.

reference:
    h     = tanh(x @ (W1+W2) + (b1+b2))      x: [B, T, D]
    score = h @ V + V_b                      [B, T, 1]
    attn  = softmax(score, axis=T)
    out   = sum_t attn * x                   [B, D]

Sharding: data-parallel over batch; each of the 8 cores gets B/8 = 8
examples (8 MB of fp32), weights replicated. No collectives.

Layout: token t = c*2048 + p*16 + i lands on partition p, tile i of
example c (token order within an example is permuted vs the reference;
softmax pooling is permutation-invariant).  Each partition reads
contiguous 2-8 KB runs from HBM.

v2 data path (vs the v1 SWDGE-cast design): x streams as plain f32
over the HARDWARE DGE (SP + ACT queues), which starts at ~4 us and
runs at HBM line rate, instead of the software-DGE cast path that was
desc-gen paced (~288 GB/s) and started at ~9 us behind the Q7
preamble.  f32 -> bf16 casts are split across DVE (c0, c1, part of
c7) and the otherwise-idle GPSIMD/Pool engine (c2..c6, rest of c7).
Identities are built on DVE so Q7 never blocks anything.

Per-core compute per 4-tile cluster: PE transposes x (xT) via the
identity; PE h-matmul with stationary Wc -> psum; ACT tanh(+bias) ->
hT sbuf; PE score matmul (stationary hT tile, 1-col rhs v) -> psum
column; ACT exp per example -> masked e-block column with accum_out
denominator; PE context matmul (stationary x tile, 8-col e-block rhs)
accumulating ctx^T [d, 8] across all 128 tiles in one psum bank.
PE emission is software-pipelined: transposes+h-matmuls of example c
run before scores of c (hiding the tanh latency), and ctx of c-1 is
interleaved into c's slot.  First and last examples run at 4-tile
granularity for ramp/tail latency; the middle at 8-tile.

V_b is omitted: softmax(score + c) == softmax(score) exactly.
Softmax runs without max-subtraction: |score| <= sum|V_u| ~ 9.
"""

import os

import numpy as np

B, T, D, U = 64, 2048, 128, 128
N_CORES = 8
B_LOC = B // N_CORES          # 8 examples per core
N_TILE_EX = T // 128          # 16 token-tiles per example
N_TILES = B_LOC * N_TILE_EX   # 128 tiles per core

_nc = None
LAST_RESULT = None


def _build_nc():
    import concourse.bass as bass  # noqa: F401
    import concourse.mybir as mybir
    import concourse.tile as tile
    from concourse import bacc

    f32 = mybir.dt.float32
    bf16 = mybir.dt.bfloat16
    ACT = mybir.ActivationFunctionType
    ALU = mybir.AluOpType

    nc = bacc.Bacc("TRN2", target_bir_lowering=False, debug=False,
                   num_devices=N_CORES)

    x_d = nc.declare_dram_parameter("encoder_outputs", [B_LOC * T, D], f32,
                                    isOutput=False)
    w1_d = nc.declare_dram_parameter("W1_w", [D, U], f32, isOutput=False)
    b1_d = nc.declare_dram_parameter("W1_b", [U, 1], f32, isOutput=False)
    w2_d = nc.declare_dram_parameter("W2_w", [D, U], f32, isOutput=False)
    b2_d = nc.declare_dram_parameter("W2_b", [U, 1], f32, isOutput=False)
    v_d = nc.declare_dram_parameter("V_w", [U, 1], f32, isOutput=False)
    out_d = nc.declare_dram_parameter("out", [B_LOC, D], f32, isOutput=True)

    # token = c*T + p*16 + i  ->  [c][p][i][d]; per-(c,p) HBM runs are 8 KB
    x_r = x_d.ap().rearrange("(c p i) d -> c p i d", c=B_LOC, p=128,
                             i=N_TILE_EX)

    def make_ident(eng, dst):
        eng.memset(dst, 0.0)
        eng.affine_select(out=dst, in_=dst, compare_op=ALU.not_equal,
                          fill=1.0, base=0, pattern=[[-1, dst.shape[1]]],
                          channel_multiplier=1)

    with tile.TileContext(nc) as tc:
        with (
            tc.tile_pool(name="consts", bufs=1) as consts,
            tc.tile_pool(name="big", bufs=1) as big,
            tc.tile_pool(name="ps_xt", bufs=2, space="PSUM") as ps_xt_pool,
            tc.tile_pool(name="ps_h", bufs=2, space="PSUM") as ps_h_pool,
            tc.tile_pool(name="ps_sc", bufs=1, space="PSUM") as ps_sc_pool,
            tc.tile_pool(name="ps_cx", bufs=1, space="PSUM") as ps_cx_pool,
        ):
            # ---- persistent buffers ----
            x_bf = big.tile([128, N_TILES * 128], bf16)     # 4 MB  [t, d]
            xT = big.tile([128, N_TILES * 128], bf16)       # 4 MB  [d, t]
            ht = big.tile([128, N_TILES * 128], bf16)       # 4 MB tanh(h)^T
            eb = big.tile([128, N_TILES * 8], bf16)         # masked e-blocks
            e_all = big.tile([128, 8], f32)
            e_7a = big.tile([128, 1], f32)
            e_7b = big.tile([128, 1], f32)
            cxT_sb = big.tile([128, 8], f32)
            out_sb = big.tile([B_LOC, 128], f32)
            den_r = big.tile([B_LOC, 1], f32)

            x_bf_r = x_bf.rearrange("p (j d) -> p j d", d=128)
            eb_r = eb.rearrange("p (j e) -> p j e", e=8)

            ident = consts.tile([128, 128], bf16)
            ident_f = consts.tile([128, 128], f32)
            w1_sb = consts.tile([128, 128], f32)
            w2_sb = consts.tile([128, 128], f32)
            wc_bf = consts.tile([128, 128], bf16)   # [d, u] stationary
            b1_sb = consts.tile([128, 1], f32)
            b2_sb = consts.tile([128, 1], f32)
            b_sum = consts.tile([128, 1], f32)      # per-partition (=u) bias
            vf_sb = consts.tile([128, 1], f32)
            v_bf = consts.tile([128, 1], bf16)
            ones_col = consts.tile([128, 1], f32)
            act_warm = consts.tile([128, 1], f32)

            ps_sc = ps_sc_pool.tile([128, 128], f32)   # score col per tile
            ps_cx = ps_cx_pool.tile([128, 512], f32)

            # ================= DMA issue =================
            # Every queue pays a ~6.5us framework preamble, so nothing moves
            # before ~7.5us.  ALL of x rides ONE SWDGE casting stream on Q7
            # in consumption order (f32 HBM -> bf16 SBUF inside the DMA
            # engines; a competing HWDGE x stream just steals engine slots
            # from the head of this one — measured in v3).  The stream leads
            # with fine c0 chunks; the bf16 identity builds while c0's first
            # bytes are in flight.
            x_chunks = [(0, 2), (2, 2), None,        # c0: 2+2, then ident
                        (4, 4), (8, 8),              # c0 rest
                        (16, 8), (24, 8)]            # c1 halves
            x_chunks += [(16 * c, 16) for c in range(2, 7)]
            # last example: 4+4+4+2+2 (ever-finer tail)
            x_chunks += [(112, 4), (116, 4), (120, 4), (124, 2), (126, 2)]
            for item in x_chunks:
                if item is None:
                    make_ident(nc.gpsimd, ident)
                    continue
                lo, w = item
                src_c, src_lo = lo // 16, lo % 16
                nc.gpsimd.dma_start(
                    out=x_bf_r[:, lo:lo + w],
                    in_=x_r[src_c][:, src_lo:src_lo + w])
            make_ident(nc.gpsimd, ident_f)   # needed only at the tail

            # weights on the SP HWDGE (idle engines before the stream arms)
            nc.sync.dma_start(out=w1_sb, in_=w1_d.ap())
            nc.sync.dma_start(out=w2_sb, in_=w2_d.ap())
            # bias/v configs on the ACT queue (behind the hoisted
            # activation-table load; land in time for first tanh/score)
            nc.scalar.dma_start(out=b1_sb, in_=b1_d.ap())
            nc.scalar.dma_start(out=b2_sb, in_=b2_d.ap())
            nc.scalar.dma_start(out=vf_sb, in_=v_d.ap())

            # scratch for the PE HAM warm-up (memset first so the dummy
            # matmuls never read uninitialized SBUF)
            scratch = consts.tile([128, 512], bf16)
            nc.vector.memset(scratch, 0.5)
            nc.vector.memset(ones_col, 1.0)
            # ACT: warm the exp/tanh table during the DMA ramp
            nc.scalar.activation(act_warm, ones_col, ACT.Exp)

            # PE warm-up: real (non-transpose) matmuls in the otherwise-dead
            # window before c0 lands, to pull the HAM full-clock flip
            # earlier.  They chain WAW on one psum tile; done before the
            # first real transpose needs the pool.
            ph_warm = ps_h_pool.tile([128, 1024], f32, tag="ph")
            for _ in range(6):
                nc.tensor.matmul(ph_warm[:, 0:512], lhsT=scratch[:, 0:128],
                                 rhs=scratch)

            nc.vector.tensor_add(wc_bf, w1_sb, w2_sb)
            nc.vector.tensor_add(b_sum, b1_sb, b2_sb)
            nc.vector.tensor_copy(v_bf, vf_sb)
            nc.vector.memset(eb, 0.0)

            # ================= main pipeline =================
            # Per example c: PE does transposes + h-matmuls for both halves,
            # then ctx(c-1), then scores(c).  ACT does exp(c-1) then tanh(c).
            def transposes(j0, w, fillers=None):
                """One w-tile cluster starting at tile j0 -> xT via psum.
                w=8 amortizes the DVE psum-drain copy; w=4 for ramp/tail.
                fillers: small ld-bound matmul closures interleaved after
                each transpose so their ldweights hide under the transpose
                streams instead of running back-to-back ld-exposed."""
                pxt = ps_xt_pool.tile([128, 128 * w], bf16, tag="pxt")
                for r in range(w):
                    nc.tensor.transpose(
                        pxt[:, 128 * r:128 * (r + 1)],
                        x_bf_r[:, j0 + r], ident)
                    if fillers:
                        k = -(-len(fillers) // (w - r))  # even spread
                        for _ in range(min(k, len(fillers))):
                            fillers.pop(0)()
                s = 128 * j0
                nc.vector.tensor_copy(xT[:, s:s + 128 * w], pxt)

            def ctx_mm_one(j):
                nc.tensor.matmul(ps_cx[:, 0:8], lhsT=x_bf_r[:, j],
                                 rhs=eb_r[:, j],
                                 start=(j == 0), stop=(j == N_TILES - 1))

            def ctx_fillers(c):
                return [(lambda j=16 * c + i: ctx_mm_one(j))
                        for i in range(N_TILE_EX)]

            def ctx_mms(c):
                for f in ctx_fillers(c):
                    f()

            def scores(j0, n):
                for i in range(n):
                    j = j0 + i
                    nc.tensor.matmul(ps_sc[:, j:j + 1],
                                     lhsT=ht[:, 128 * j:128 * (j + 1)],
                                     rhs=v_bf)

            def exp_c(c):
                nc.scalar.activation(eb_r[:, 16 * c:16 * c + 16, c],
                                     ps_sc[:, 16 * c:16 * c + 16],
                                     ACT.Exp, accum_out=e_all[:, c:c + 1])

            def h_mm(ph, base, lo, hi):
                nc.tensor.matmul(ph[:, lo:hi], lhsT=wc_bf,
                                 rhs=xT[:, base + lo:base + hi])

            def warm_mm():
                nc.tensor.matmul(ph_warm[:, 512:640],
                                 lhsT=scratch[:, 0:128],
                                 rhs=scratch[:, 0:128])

            for c in range(B_LOC - 1):
                if c >= 1:
                    # ACT queue: previous example's exp runs while PE does
                    # this example's transposes/h-matmuls
                    exp_c(c - 1)
                # ctx(c-1) matmuls ride inside this example's SECOND
                # transpose cluster (2 per transpose; their ldweights hide
                # under the transpose streams, and by then exp(c-1) is
                # done).  For c0 the fillers are HAM-warming dummies that
                # also fill the early stream-wait gaps.
                fillers = ctx_fillers(c - 1) if c >= 1 else \
                    [warm_mm for _ in range(16)]
                for g in range(2):
                    base = 2048 * c + 1024 * g
                    j0 = 16 * c + 8 * g
                    ph = ps_h_pool.tile([128, 1024], f32, tag="ph")
                    fill = fillers if (c == 0 or g == 1) else None
                    if c == 0:   # 4-tile chains for the ramp
                        half = fillers[:8]
                        del fillers[:8]
                        transposes(j0, 4, half)
                        h_mm(ph, base, 0, 512)
                        nc.scalar.activation(ht[:, base:base + 512],
                                             ph[:, 0:512], ACT.Tanh,
                                             bias=b_sum)
                        transposes(j0 + 4, 4, half)
                        h_mm(ph, base, 512, 1024)
                        nc.scalar.activation(ht[:, base + 512:base + 1024],
                                             ph[:, 512:1024], ACT.Tanh,
                                             bias=b_sum)
                    else:
                        transposes(j0, 8, fill)
                        h_mm(ph, base, 0, 512)
                        h_mm(ph, base, 512, 1024)
                        nc.scalar.activation(ht[:, base:base + 1024], ph,
                                             ACT.Tanh, bias=b_sum)
                scores(16 * c, 8)
                scores(16 * c + 8, 8)

            # ---- last example: fully pipelined ever-finer tail ----
            # chunks of 4,4,4,2,2 tiles; tanh/scores/exp/ctx per chunk
            c = B_LOC - 1
            CH = [(112, 4), (116, 4), (120, 4), (124, 2), (126, 2)]
            e7q = [e_7a, e_7b,
                   big.tile([128, 1], f32, name="e_7c"),
                   big.tile([128, 1], f32, name="e_7d"),
                   big.tile([128, 1], f32, name="e_7e")]
            # ph tiles: chunks 0-1 share ph0; chunks 2-4 share ph1
            ph7 = [None, None]
            ph_of = [(0, 0), (0, 512), (1, 0), (1, 512), (1, 768)]

            def c7_chunk(q):
                j0, w = CH[q]
                t, lo = ph_of[q]
                if ph7[t] is None or (t, lo) in ((0, 0), (1, 0)):
                    ph7[t] = ps_h_pool.tile([128, 1024], f32, tag="ph",
                                            name="ph7")
                ph = ph7[t]
                transposes(j0, w)
                cols = 128 * w
                nc.tensor.matmul(ph[:, lo:lo + cols], lhsT=wc_bf,
                                 rhs=xT[:, 128 * j0:128 * j0 + cols])
                nc.scalar.activation(ht[:, 128 * j0:128 * j0 + cols],
                                     ph[:, lo:lo + cols], ACT.Tanh,
                                     bias=b_sum)

            def sc7(q):
                scores(CH[q][0], CH[q][1])

            def exp7(q):
                j0, w = CH[q]
                nc.scalar.activation(eb_r[:, j0:j0 + w, c],
                                     ps_sc[:, j0:j0 + w],
                                     ACT.Exp, accum_out=e7q[q])

            def ctx7(q):
                j0, w = CH[q]
                for i in range(w):
                    j = j0 + i
                    nc.tensor.matmul(ps_cx[:, 0:8], lhsT=x_bf_r[:, j],
                                     rhs=eb_r[:, j],
                                     start=(j == 0), stop=(j == N_TILES - 1))

            exp_c(c - 1)
            c7_chunk(0)
            c7_chunk(1)
            ctx_mms(c - 1)
            sc7(0); exp7(0)
            c7_chunk(2)
            sc7(1); exp7(1)
            ctx7(0)
            c7_chunk(3)
            sc7(2); exp7(2)
            ctx7(1)
            c7_chunk(4)
            sc7(3); exp7(3)
            ctx7(2)
            sc7(4); exp7(4)
            ctx7(3)
            ctx7(4)

            # ---- denominator + final transpose/scale ----
            s01 = big.tile([128, 1], f32)
            s23 = big.tile([128, 1], f32)
            s04 = big.tile([128, 1], f32)
            nc.vector.tensor_add(s01, e7q[0], e7q[1])
            nc.vector.tensor_add(s23, e7q[2], e7q[3])
            nc.vector.tensor_add(s04, s01, s23)
            nc.vector.tensor_add(e_all[:, c:c + 1], s04, e7q[4])
            nc.tensor.matmul(ps_cx[0:8, 448:449], lhsT=e_all, rhs=ones_col)

            nc.vector.tensor_copy(cxT_sb, ps_cx[:, 0:8])
            nc.tensor.transpose(ps_cx[0:8, 320:448], cxT_sb, ident_f)

            nc.vector.reciprocal(den_r, ps_cx[0:8, 448:449])
            nc.vector.tensor_scalar_mul(out_sb, ps_cx[0:8, 320:448], den_r)
            nc.sync.dma_start(out=out_d.ap(), in_=out_sb)

    nc.compile()
    return nc


def get_nc():
    global _nc
    if _nc is None:
        _nc = _build_nc()
    return _nc


def kernel(encoder_outputs, W1_w, W1_b, W2_w, W2_b, V_w, V_b):
    global LAST_RESULT
    from concourse.bass_utils import run_bass_kernel_spmd

    nc = get_nc()

    enc = np.ascontiguousarray(np.asarray(encoder_outputs, dtype=np.float32))
    rep = {
        "W1_w": np.ascontiguousarray(np.asarray(W1_w, np.float32)),
        "W1_b": np.ascontiguousarray(np.asarray(W1_b, np.float32).reshape(U, 1)),
        "W2_w": np.ascontiguousarray(np.asarray(W2_w, np.float32)),
        "W2_b": np.ascontiguousarray(np.asarray(W2_b, np.float32).reshape(U, 1)),
        "V_w": np.ascontiguousarray(np.asarray(V_w, np.float32).reshape(U, 1)),
    }
    in_maps = []
    for c in range(N_CORES):
        shard = enc[c * B_LOC:(c + 1) * B_LOC].reshape(B_LOC * T, D)
        in_maps.append({"encoder_outputs": np.ascontiguousarray(shard), **rep})

    trace = bool(int(os.environ.get("KERNEL_TRACE", "0")))
    LAST_RESULT = run_bass_kernel_spmd(
        nc, in_maps, core_ids=list(range(N_CORES)), trace=trace)
    out = np.concatenate(
        [LAST_RESULT.results[c]["out"] for c in range(N_CORES)], axis=0)
    return np.ascontiguousarray(out, dtype=np.float32)


# revision 40
# speedup vs baseline: 1.0937x; 1.0082x over previous
"""Additive-attention pooling kernel for 8 TRN2 NeuronCores.

reference:
    h     = tanh(x @ (W1+W2) + (b1+b2))      x: [B, T, D]
    score = h @ V + V_b                      [B, T, 1]
    attn  = softmax(score, axis=T)
    out   = sum_t attn * x                   [B, D]

Sharding: data-parallel over batch; each of the 8 cores gets B/8 = 8
examples (8 MB of fp32), weights replicated. No collectives.

Layout: token t = c*2048 + p*16 + i lands on partition p, tile i of
example c (token order within an example is permuted vs the reference;
softmax pooling is permutation-invariant).  Each partition reads
contiguous 2-8 KB runs from HBM.

v2 data path (vs the v1 SWDGE-cast design): x streams as plain f32
over the HARDWARE DGE (SP + ACT queues), which starts at ~4 us and
runs at HBM line rate, instead of the software-DGE cast path that was
desc-gen paced (~288 GB/s) and started at ~9 us behind the Q7
preamble.  f32 -> bf16 casts are split across DVE (c0, c1, part of
c7) and the otherwise-idle GPSIMD/Pool engine (c2..c6, rest of c7).
Identities are built on DVE so Q7 never blocks anything.

Per-core compute per 4-tile cluster: PE transposes x (xT) via the
identity; PE h-matmul with stationary Wc -> psum; ACT tanh(+bias) ->
hT sbuf; PE score matmul (stationary hT tile, 1-col rhs v) -> psum
column; ACT exp per example -> masked e-block column with accum_out
denominator; PE context matmul (stationary x tile, 8-col e-block rhs)
accumulating ctx^T [d, 8] across all 128 tiles in one psum bank.
PE emission is software-pipelined: transposes+h-matmuls of example c
run before scores of c (hiding the tanh latency), and ctx of c-1 is
interleaved into c's slot.  First and last examples run at 4-tile
granularity for ramp/tail latency; the middle at 8-tile.

V_b is omitted: softmax(score + c) == softmax(score) exactly.
Softmax runs without max-subtraction: |score| <= sum|V_u| ~ 9.
"""

import os

import numpy as np

B, T, D, U = 64, 2048, 128, 128
N_CORES = 8
B_LOC = B // N_CORES          # 8 examples per core
N_TILE_EX = T // 128          # 16 token-tiles per example
N_TILES = B_LOC * N_TILE_EX   # 128 tiles per core

_nc = None
LAST_RESULT = None


def _build_nc():
    import concourse.bass as bass  # noqa: F401
    import concourse.mybir as mybir
    import concourse.tile as tile
    from concourse import bacc

    f32 = mybir.dt.float32
    bf16 = mybir.dt.bfloat16
    ACT = mybir.ActivationFunctionType
    ALU = mybir.AluOpType

    nc = bacc.Bacc("TRN2", target_bir_lowering=False, debug=False,
                   num_devices=N_CORES)

    x_d = nc.declare_dram_parameter("encoder_outputs", [B_LOC * T, D], f32,
                                    isOutput=False)
    w1_d = nc.declare_dram_parameter("W1_w", [D, U], f32, isOutput=False)
    b1_d = nc.declare_dram_parameter("W1_b", [U, 1], f32, isOutput=False)
    w2_d = nc.declare_dram_parameter("W2_w", [D, U], f32, isOutput=False)
    b2_d = nc.declare_dram_parameter("W2_b", [U, 1], f32, isOutput=False)
    v_d = nc.declare_dram_parameter("V_w", [U, 1], f32, isOutput=False)
    out_d = nc.declare_dram_parameter("out", [B_LOC, D], f32, isOutput=True)

    # token = c*T + p*16 + i  ->  [c][p][i][d]; per-(c,p) HBM runs are 8 KB
    x_r = x_d.ap().rearrange("(c p i) d -> c p i d", c=B_LOC, p=128,
                             i=N_TILE_EX)

    def make_ident(eng, dst):
        eng.memset(dst, 0.0)
        eng.affine_select(out=dst, in_=dst, compare_op=ALU.not_equal,
                          fill=1.0, base=0, pattern=[[-1, dst.shape[1]]],
                          channel_multiplier=1)

    with tile.TileContext(nc) as tc:
        with (
            tc.tile_pool(name="consts", bufs=1) as consts,
            tc.tile_pool(name="big", bufs=1) as big,
            tc.tile_pool(name="ps_xt", bufs=2, space="PSUM") as ps_xt_pool,
            tc.tile_pool(name="ps_h", bufs=2, space="PSUM") as ps_h_pool,
            tc.tile_pool(name="ps_sc", bufs=1, space="PSUM") as ps_sc_pool,
            tc.tile_pool(name="ps_cx", bufs=1, space="PSUM") as ps_cx_pool,
        ):
            # ---- persistent buffers ----
            x_bf = big.tile([128, N_TILES * 128], bf16)     # 4 MB  [t, d]
            xT = big.tile([128, N_TILES * 128], bf16)       # 4 MB  [d, t]
            ht = big.tile([128, N_TILES * 128], bf16)       # 4 MB tanh(h)^T
            eb = big.tile([128, N_TILES * 8], bf16)         # masked e-blocks
            e_all = big.tile([128, 8], f32)
            e_7a = big.tile([128, 1], f32)
            e_7b = big.tile([128, 1], f32)
            cxT_sb = big.tile([128, 8], f32)
            out_sb = big.tile([B_LOC, 128], f32)
            den_r = big.tile([B_LOC, 1], f32)

            x_bf_r = x_bf.rearrange("p (j d) -> p j d", d=128)
            eb_r = eb.rearrange("p (j e) -> p j e", e=8)

            ident = consts.tile([128, 128], bf16)
            ident_f = consts.tile([128, 128], f32)
            w1_sb = consts.tile([128, 128], f32)
            w2_sb = consts.tile([128, 128], f32)
            wc_bf = consts.tile([128, 128], bf16)   # [d, u] stationary
            b1_sb = consts.tile([128, 1], f32)
            b2_sb = consts.tile([128, 1], f32)
            b_sum = consts.tile([128, 1], f32)      # per-partition (=u) bias
            vf_sb = consts.tile([128, 1], f32)
            v_bf = consts.tile([128, 1], bf16)
            ones_col = consts.tile([128, 1], f32)
            act_warm = consts.tile([128, 1], f32)

            ps_sc = ps_sc_pool.tile([128, 128], f32)   # score col per tile
            ps_cx = ps_cx_pool.tile([128, 512], f32)

            # ================= DMA issue =================
            # Every queue pays a ~6.5us framework preamble, so nothing moves
            # before ~7.5us.  ALL of x rides ONE SWDGE casting stream on Q7
            # in consumption order (f32 HBM -> bf16 SBUF inside the DMA
            # engines; a competing HWDGE x stream just steals engine slots
            # from the head of this one — measured in v3).  The stream leads
            # with fine c0 chunks; the bf16 identity builds while c0's first
            # bytes are in flight.
            x_chunks = [(0, 2), (2, 2), None,        # c0: 2+2, then ident
                        (4, 4), (8, 8),              # c0 rest
                        (16, 8), (24, 8)]            # c1 halves
            x_chunks += [(16 * c, 16) for c in range(2, 7)]
            # last example: 4+4+4+2+2 (ever-finer tail)
            x_chunks += [(112, 4), (116, 4), (120, 4), (124, 2), (126, 2)]
            for item in x_chunks:
                if item is None:
                    make_ident(nc.gpsimd, ident)
                    continue
                lo, w = item
                src_c, src_lo = lo // 16, lo % 16
                nc.gpsimd.dma_start(
                    out=x_bf_r[:, lo:lo + w],
                    in_=x_r[src_c][:, src_lo:src_lo + w])
            make_ident(nc.gpsimd, ident_f)   # needed only at the tail

            # weights on the SP HWDGE (idle engines before the stream arms)
            nc.sync.dma_start(out=w1_sb, in_=w1_d.ap())
            nc.sync.dma_start(out=w2_sb, in_=w2_d.ap())
            # bias/v configs on the ACT queue (behind the hoisted
            # activation-table load; land in time for first tanh/score)
            nc.scalar.dma_start(out=b1_sb, in_=b1_d.ap())
            nc.scalar.dma_start(out=b2_sb, in_=b2_d.ap())
            nc.scalar.dma_start(out=vf_sb, in_=v_d.ap())

            # scratch for the PE HAM warm-up (memset first so the dummy
            # matmuls never read uninitialized SBUF)
            scratch = consts.tile([128, 512], bf16)
            nc.vector.memset(scratch, 0.5)
            nc.vector.memset(ones_col, 1.0)
            # ACT: warm the exp/tanh table during the DMA ramp
            nc.scalar.activation(act_warm, ones_col, ACT.Exp)

            # PE warm-up: real (non-transpose) matmuls in the otherwise-dead
            # window before c0 lands, to pull the HAM full-clock flip
            # earlier.  They chain WAW on one psum tile; done before the
            # first real transpose needs the pool.
            ph_warm = ps_h_pool.tile([128, 1024], f32, tag="ph")
            for _ in range(6):
                nc.tensor.matmul(ph_warm[:, 0:512], lhsT=scratch[:, 0:128],
                                 rhs=scratch)

            nc.vector.tensor_add(wc_bf, w1_sb, w2_sb)
            nc.vector.tensor_add(b_sum, b1_sb, b2_sb)
            nc.vector.tensor_copy(v_bf, vf_sb)
            nc.vector.memset(eb, 0.0)

            # ================= main pipeline =================
            # Per example c: PE does transposes + h-matmuls for both halves,
            # then ctx(c-1), then scores(c).  ACT does exp(c-1) then tanh(c).
            def transposes(j0, w, fillers=None):
                """One w-tile cluster starting at tile j0 -> xT via psum.
                w=8 amortizes the DVE psum-drain copy; w=4 for ramp/tail.
                fillers: small ld-bound matmul closures interleaved after
                each transpose so their ldweights hide under the transpose
                streams instead of running back-to-back ld-exposed."""
                pxt = ps_xt_pool.tile([128, 128 * w], bf16, tag="pxt")
                for r in range(w):
                    nc.tensor.transpose(
                        pxt[:, 128 * r:128 * (r + 1)],
                        x_bf_r[:, j0 + r], ident)
                    if fillers:
                        k = -(-len(fillers) // (w - r))  # even spread
                        for _ in range(min(k, len(fillers))):
                            fillers.pop(0)()
                s = 128 * j0
                nc.vector.tensor_copy(xT[:, s:s + 128 * w], pxt)

            def ctx_mm_one(j):
                nc.tensor.matmul(ps_cx[:, 0:8], lhsT=x_bf_r[:, j],
                                 rhs=eb_r[:, j],
                                 start=(j == 0), stop=(j == N_TILES - 1))

            def ctx_fillers(c):
                return [(lambda j=16 * c + i: ctx_mm_one(j))
                        for i in range(N_TILE_EX)]

            def ctx_mms(c):
                for f in ctx_fillers(c):
                    f()

            def scores(j0, n):
                for i in range(n):
                    j = j0 + i
                    nc.tensor.matmul(ps_sc[:, j:j + 1],
                                     lhsT=ht[:, 128 * j:128 * (j + 1)],
                                     rhs=v_bf)

            def exp_c(c):
                nc.scalar.activation(eb_r[:, 16 * c:16 * c + 16, c],
                                     ps_sc[:, 16 * c:16 * c + 16],
                                     ACT.Exp, accum_out=e_all[:, c:c + 1])

            def h_mm(ph, base, lo, hi):
                nc.tensor.matmul(ph[:, lo:hi], lhsT=wc_bf,
                                 rhs=xT[:, base + lo:base + hi])

            for c in range(B_LOC - 1):
                if c >= 1:
                    # ACT queue: previous example's exp runs while PE does
                    # this example's transposes/h-matmuls
                    exp_c(c - 1)
                # ctx(c-1) matmuls ride inside this example's SECOND
                # transpose cluster (2 per transpose; their ldweights hide
                # under the transpose streams, and by then exp(c-1) is
                # done).
                fillers = ctx_fillers(c - 1) if c >= 1 else []
                for g in range(2):
                    base = 2048 * c + 1024 * g
                    j0 = 16 * c + 8 * g
                    ph = ps_h_pool.tile([128, 1024], f32, tag="ph")
                    fill = fillers if g == 1 else None
                    if c == 0:   # 4-tile chains for the ramp
                        transposes(j0, 4, None)
                        h_mm(ph, base, 0, 512)
                        nc.scalar.activation(ht[:, base:base + 512],
                                             ph[:, 0:512], ACT.Tanh,
                                             bias=b_sum)
                        transposes(j0 + 4, 4, None)
                        h_mm(ph, base, 512, 1024)
                        nc.scalar.activation(ht[:, base + 512:base + 1024],
                                             ph[:, 512:1024], ACT.Tanh,
                                             bias=b_sum)
                    else:
                        transposes(j0, 8, fill)
                        h_mm(ph, base, 0, 512)
                        h_mm(ph, base, 512, 1024)
                        nc.scalar.activation(ht[:, base:base + 1024], ph,
                                             ACT.Tanh, bias=b_sum)
                scores(16 * c, 8)
                scores(16 * c + 8, 8)

            # ---- last example: fully pipelined ever-finer tail ----
            # chunks of 4,4,4,2,2 tiles; tanh/scores/exp/ctx per chunk
            c = B_LOC - 1
            CH = [(112, 4), (116, 4), (120, 4), (124, 2), (126, 2)]
            e7q = [e_7a, e_7b,
                   big.tile([128, 1], f32, name="e_7c"),
                   big.tile([128, 1], f32, name="e_7d"),
                   big.tile([128, 1], f32, name="e_7e")]
            # ph tiles: chunks 0-1 share ph0; chunks 2-4 share ph1
            ph7 = [None, None]
            ph_of = [(0, 0), (0, 512), (1, 0), (1, 512), (1, 768)]

            def c7_chunk(q):
                j0, w = CH[q]
                t, lo = ph_of[q]
                if ph7[t] is None or (t, lo) in ((0, 0), (1, 0)):
                    ph7[t] = ps_h_pool.tile([128, 1024], f32, tag="ph",
                                            name="ph7")
                ph = ph7[t]
                transposes(j0, w)
                cols = 128 * w
                nc.tensor.matmul(ph[:, lo:lo + cols], lhsT=wc_bf,
                                 rhs=xT[:, 128 * j0:128 * j0 + cols])
                nc.scalar.activation(ht[:, 128 * j0:128 * j0 + cols],
                                     ph[:, lo:lo + cols], ACT.Tanh,
                                     bias=b_sum)

            def sc7(q):
                scores(CH[q][0], CH[q][1])

            def exp7(q):
                j0, w = CH[q]
                nc.scalar.activation(eb_r[:, j0:j0 + w, c],
                                     ps_sc[:, j0:j0 + w],
                                     ACT.Exp, accum_out=e7q[q])

            def ctx7(q):
                j0, w = CH[q]
                for i in range(w):
                    j = j0 + i
                    nc.tensor.matmul(ps_cx[:, 0:8], lhsT=x_bf_r[:, j],
                                     rhs=eb_r[:, j],
                                     start=(j == 0), stop=(j == N_TILES - 1))

            exp_c(c - 1)
            c7_chunk(0)
            c7_chunk(1)
            ctx_mms(c - 1)
            sc7(0); exp7(0)
            c7_chunk(2)
            sc7(1); exp7(1)
            ctx7(0)
            c7_chunk(3)
            sc7(2); exp7(2)
            ctx7(1)
            c7_chunk(4)
            sc7(3); exp7(3)
            ctx7(2)
            sc7(4); exp7(4)
            ctx7(3)
            ctx7(4)

            # ---- denominator + final transpose/scale ----
            s01 = big.tile([128, 1], f32)
            s23 = big.tile([128, 1], f32)
            s04 = big.tile([128, 1], f32)
            nc.vector.tensor_add(s01, e7q[0], e7q[1])
            nc.vector.tensor_add(s23, e7q[2], e7q[3])
            nc.vector.tensor_add(s04, s01, s23)
            nc.vector.tensor_add(e_all[:, c:c + 1], s04, e7q[4])
            nc.tensor.matmul(ps_cx[0:8, 448:449], lhsT=e_all, rhs=ones_col)

            nc.vector.tensor_copy(cxT_sb, ps_cx[:, 0:8])
            nc.tensor.transpose(ps_cx[0:8, 320:448], cxT_sb, ident_f)

            nc.vector.reciprocal(den_r, ps_cx[0:8, 448:449])
            nc.vector.tensor_scalar_mul(out_sb, ps_cx[0:8, 320:448], den_r)
            nc.sync.dma_start(out=out_d.ap(), in_=out_sb)

    nc.compile()
    return nc


def get_nc():
    global _nc
    if _nc is None:
        _nc = _build_nc()
    return _nc


def kernel(encoder_outputs, W1_w, W1_b, W2_w, W2_b, V_w, V_b):
    global LAST_RESULT
    from concourse.bass_utils import run_bass_kernel_spmd

    nc = get_nc()

    enc = np.ascontiguousarray(np.asarray(encoder_outputs, dtype=np.float32))
    rep = {
        "W1_w": np.ascontiguousarray(np.asarray(W1_w, np.float32)),
        "W1_b": np.ascontiguousarray(np.asarray(W1_b, np.float32).reshape(U, 1)),
        "W2_w": np.ascontiguousarray(np.asarray(W2_w, np.float32)),
        "W2_b": np.ascontiguousarray(np.asarray(W2_b, np.float32).reshape(U, 1)),
        "V_w": np.ascontiguousarray(np.asarray(V_w, np.float32).reshape(U, 1)),
    }
    in_maps = []
    for c in range(N_CORES):
        shard = enc[c * B_LOC:(c + 1) * B_LOC].reshape(B_LOC * T, D)
        in_maps.append({"encoder_outputs": np.ascontiguousarray(shard), **rep})

    trace = bool(int(os.environ.get("KERNEL_TRACE", "0")))
    LAST_RESULT = run_bass_kernel_spmd(
        nc, in_maps, core_ids=list(range(N_CORES)), trace=trace)
    out = np.concatenate(
        [LAST_RESULT.results[c]["out"] for c in range(N_CORES)], axis=0)
    return np.ascontiguousarray(out, dtype=np.float32)


# revision 41
# speedup vs baseline: 1.1106x; 1.0155x over previous
"""Additive-attention pooling kernel for 8 TRN2 NeuronCores.

reference:
    h     = tanh(x @ (W1+W2) + (b1+b2))      x: [B, T, D]
    score = h @ V + V_b                      [B, T, 1]
    attn  = softmax(score, axis=T)
    out   = sum_t attn * x                   [B, D]

Sharding: data-parallel over batch; each of the 8 cores gets B/8 = 8
examples (8 MB of fp32), weights replicated. No collectives.

Layout: token t = c*2048 + p*16 + i lands on partition p, tile i of
example c (token order within an example is permuted vs the reference;
softmax pooling is permutation-invariant), so each (example,
partition) pair is one contiguous 8 KB HBM run.

Data path: ALL of x rides ONE SWDGE casting stream on Q7 (f32 HBM ->
bf16 SBUF inside the DMA engines), issued in consumption order:
c0 in 2+2+4+8-tile chunks (the bf16 identity build is slotted after
the first two calls), c1 halves, c2..c6 whole, c7 in 4+4+4+2+2 (ever-
finer tail).  Weights ride the SP HWDGE, bias/v the ACT-queue HWDGE.

Per-core compute: PE transposes x tiles (xT) via the identity into
bf16 psum; DVE drains psum->sbuf; PE h-matmul (stationary Wc, 512-col
xT rhs) -> psum; ACT tanh(+per-partition bias) -> hT sbuf; PE score
matmul per tile (stationary hT, 1-col v rhs) -> score column; ACT exp
per example -> masked e-block column + accum_out denominator; PE ctx
matmul (stationary x tile, 8-col e-block rhs) accumulates ctx^T
[d, 8] across all 128 tiles in one psum bank.  Emission is software-
pipelined: ACT order per slot is [exp(c-1), tanh(c, g0), tanh(c, g1)]
so PE's ctx(c-1) never waits; ctx(c-1) matmuls ride INSIDE example
c's second transpose cluster (2 per transpose — their ldweights hide
under the 128-row transpose streams); scores(c) follow.  c7 pipelines
tanh/scores/exp/ctx per 4..2-tile chunk for the shortest tail.
Six dummy 512-col matmuls on scratch run in the dead window before c0
lands: they pull the HAM full-clock flip from ~31 us (cold) to
~11-19 us.

Measured facts (axon TRN2 fleet, from NTFF traces — do not re-learn):
 - Every engine queue pays ~6.5 us of framework preamble (barrier +
   DSP program load); first DMA bytes move ~8.6 us.  Fixed.
 - Aggregate DMA is ~330 GB/s for this 8 KB-descriptor mix no matter
   how traffic is split across SWDGE/HWDGE; a parallel HWDGE x stream
   just steals engine slots from the SWDGE head (v3 regression), and
   HWDGE x for mid-examples is ~neutral (v6).  Stream spans
   ~[8.6, 35] us.
 - GPSIMD/Pool tensor ops run ~3.7 ns/elem/lane (~4.5x worse than the
   cost model) — never cast there (v2 regression, 42 us busy).
 - PE: bf16 transposes/matmuls stream 1 col/cycle, ldweights 2
   rows/cycle and pipeline under the previous stream (ctx+score
   matmuls run back-to-back at ~30 ns each).  PE stream work here is
   33.9k rows ~= 14.1 us at 2.4 GHz, but HAM starts the PE at
   0.65-1.2 GHz and duty-cycles k=4/8 under power contention, so
   PE busy measures 29-33 us.  PE is the end-to-end pacer.
 - exp/tanh share one ACT table set (no reload cost); ACT busy ~26 us
   (tanh floor 13.7); DVE ~15 (xT copies pace transposes, bufs=2).
 - Exec: 50.8-52 us contended (min of 3, spread between invocations
   49.8-59.6; within-invocation spread ~1.5%).  Baseline v1 measured
   54.1 us in the same conditions.

Dead ends (proved, do not retry): any transpose-free h formulation
(PE contracts partitions; x lands token-major), DoubleRow fp8
(needs an interleaved K-layout no engine can produce), DVE
StreamTranspose (32x32 blocks -> 16 instrs/tile), dma_start_transpose
(per-call overhead), psum-resident matmul operands (SBUF-only),
f32r compute paths (psum f32 copies at 1x kill DVE).

V_b is omitted: softmax(score + c) == softmax(score) exactly.
Softmax runs without max-subtraction: |score| <= sum|V_u| ~ 9.
"""

import os

import numpy as np

B, T, D, U = 64, 2048, 128, 128
N_CORES = 8
B_LOC = B // N_CORES          # 8 examples per core
N_TILE_EX = T // 128          # 16 token-tiles per example
N_TILES = B_LOC * N_TILE_EX   # 128 tiles per core

_nc = None
LAST_RESULT = None


def _build_nc():
    import concourse.bass as bass  # noqa: F401
    import concourse.mybir as mybir
    import concourse.tile as tile
    from concourse import bacc

    f32 = mybir.dt.float32
    bf16 = mybir.dt.bfloat16
    ACT = mybir.ActivationFunctionType
    ALU = mybir.AluOpType

    nc = bacc.Bacc("TRN2", target_bir_lowering=False, debug=False,
                   num_devices=N_CORES)

    x_d = nc.declare_dram_parameter("encoder_outputs", [B_LOC * T, D], f32,
                                    isOutput=False)
    w1_d = nc.declare_dram_parameter("W1_w", [D, U], f32, isOutput=False)
    b1_d = nc.declare_dram_parameter("W1_b", [U, 1], f32, isOutput=False)
    w2_d = nc.declare_dram_parameter("W2_w", [D, U], f32, isOutput=False)
    b2_d = nc.declare_dram_parameter("W2_b", [U, 1], f32, isOutput=False)
    v_d = nc.declare_dram_parameter("V_w", [U, 1], f32, isOutput=False)
    out_d = nc.declare_dram_parameter("out", [B_LOC, D], f32, isOutput=True)

    # token = c*T + p*16 + i  ->  [c][p][i][d]; per-(c,p) HBM runs are 8 KB
    x_r = x_d.ap().rearrange("(c p i) d -> c p i d", c=B_LOC, p=128,
                             i=N_TILE_EX)

    def make_ident(eng, dst):
        eng.memset(dst, 0.0)
        eng.affine_select(out=dst, in_=dst, compare_op=ALU.not_equal,
                          fill=1.0, base=0, pattern=[[-1, dst.shape[1]]],
                          channel_multiplier=1)

    with tile.TileContext(nc) as tc:
        with (
            tc.tile_pool(name="consts", bufs=1) as consts,
            tc.tile_pool(name="big", bufs=1) as big,
            tc.tile_pool(name="ps_xt", bufs=2, space="PSUM") as ps_xt_pool,
            tc.tile_pool(name="ps_h", bufs=2, space="PSUM") as ps_h_pool,
            tc.tile_pool(name="ps_sc", bufs=1, space="PSUM") as ps_sc_pool,
            tc.tile_pool(name="ps_cx", bufs=1, space="PSUM") as ps_cx_pool,
        ):
            # ---- persistent buffers ----
            x_bf = big.tile([128, N_TILES * 128], bf16)     # 4 MB  [t, d]
            xT = big.tile([128, N_TILES * 128], bf16)       # 4 MB  [d, t]
            ht = big.tile([128, N_TILES * 128], bf16)       # 4 MB tanh(h)^T
            eb = big.tile([128, N_TILES * 8], bf16)         # masked e-blocks
            e_all = big.tile([128, 8], f32)
            e_7a = big.tile([128, 1], f32)
            e_7b = big.tile([128, 1], f32)
            cxT_sb = big.tile([128, 8], f32)
            out_sb = big.tile([B_LOC, 128], f32)
            den_r = big.tile([B_LOC, 1], f32)

            x_bf_r = x_bf.rearrange("p (j d) -> p j d", d=128)
            eb_r = eb.rearrange("p (j e) -> p j e", e=8)

            ident = consts.tile([128, 128], bf16)
            ident_f = consts.tile([128, 128], f32)
            w1_sb = consts.tile([128, 128], f32)
            w2_sb = consts.tile([128, 128], f32)
            wc_bf = consts.tile([128, 128], bf16)   # [d, u] stationary
            b1_sb = consts.tile([128, 1], f32)
            b2_sb = consts.tile([128, 1], f32)
            b_sum = consts.tile([128, 1], f32)      # per-partition (=u) bias
            vf_sb = consts.tile([128, 1], f32)
            v_bf = consts.tile([128, 1], bf16)
            ones_col = consts.tile([128, 1], f32)
            act_warm = consts.tile([128, 1], f32)

            ps_sc = ps_sc_pool.tile([128, 128], f32)   # score col per tile
            ps_cx = ps_cx_pool.tile([128, 512], f32)

            # ================= DMA issue =================
            # Every queue pays a ~6.5us framework preamble, so nothing moves
            # before ~7.5us.  ALL of x rides ONE SWDGE casting stream on Q7
            # in consumption order (f32 HBM -> bf16 SBUF inside the DMA
            # engines; a competing HWDGE x stream just steals engine slots
            # from the head of this one — measured in v3).  The stream leads
            # with fine c0 chunks; the bf16 identity builds while c0's first
            # bytes are in flight.
            x_chunks = [(0, 2), (2, 2), None,        # c0: 2+2, then ident
                        (4, 4), (8, 8),              # c0 rest
                        (16, 8), (24, 8)]            # c1 halves
            x_chunks += [(16 * c, 16) for c in range(2, 7)]
            # last example: 4+4+4+2+2 (ever-finer tail)
            x_chunks += [(112, 4), (116, 4), (120, 4), (124, 2), (126, 2)]
            for item in x_chunks:
                if item is None:
                    make_ident(nc.gpsimd, ident)
                    continue
                lo, w = item
                src_c, src_lo = lo // 16, lo % 16
                nc.gpsimd.dma_start(
                    out=x_bf_r[:, lo:lo + w],
                    in_=x_r[src_c][:, src_lo:src_lo + w])
            make_ident(nc.gpsimd, ident_f)   # needed only at the tail

            # weights on the SP HWDGE (idle engines before the stream arms)
            nc.sync.dma_start(out=w1_sb, in_=w1_d.ap())
            nc.sync.dma_start(out=w2_sb, in_=w2_d.ap())
            # bias/v configs on the ACT queue (behind the hoisted
            # activation-table load; land in time for first tanh/score)
            nc.scalar.dma_start(out=b1_sb, in_=b1_d.ap())
            nc.scalar.dma_start(out=b2_sb, in_=b2_d.ap())
            nc.scalar.dma_start(out=vf_sb, in_=v_d.ap())

            # scratch for the PE HAM warm-up (memset first so the dummy
            # matmuls never read uninitialized SBUF)
            scratch = consts.tile([128, 512], bf16)
            nc.vector.memset(scratch, 0.5)
            nc.vector.memset(ones_col, 1.0)
            # ACT: warm the exp/tanh table during the DMA ramp
            nc.scalar.activation(act_warm, ones_col, ACT.Exp)

            # PE warm-up: real (non-transpose) matmuls in the otherwise-dead
            # window before c0 lands, to pull the HAM full-clock flip
            # earlier.  They chain WAW on one psum tile; done before the
            # first real transpose needs the pool.
            ph_warm = ps_h_pool.tile([128, 1024], f32, tag="ph")
            for _ in range(6):
                nc.tensor.matmul(ph_warm[:, 0:512], lhsT=scratch[:, 0:128],
                                 rhs=scratch)

            nc.vector.tensor_add(wc_bf, w1_sb, w2_sb)
            nc.vector.tensor_add(b_sum, b1_sb, b2_sb)
            nc.vector.tensor_copy(v_bf, vf_sb)
            nc.vector.memset(eb, 0.0)

            # ================= main pipeline =================
            # Per example c: PE does transposes + h-matmuls for both halves,
            # then ctx(c-1), then scores(c).  ACT does exp(c-1) then tanh(c).
            def transposes(j0, w, fillers=None):
                """One w-tile cluster starting at tile j0 -> xT via psum.
                w=8 amortizes the DVE psum-drain copy; w=4 for ramp/tail.
                fillers: small ld-bound matmul closures interleaved after
                each transpose so their ldweights hide under the transpose
                streams instead of running back-to-back ld-exposed."""
                pxt = ps_xt_pool.tile([128, 128 * w], bf16, tag="pxt")
                for r in range(w):
                    nc.tensor.transpose(
                        pxt[:, 128 * r:128 * (r + 1)],
                        x_bf_r[:, j0 + r], ident)
                    if fillers:
                        k = -(-len(fillers) // (w - r))  # even spread
                        for _ in range(min(k, len(fillers))):
                            fillers.pop(0)()
                s = 128 * j0
                nc.vector.tensor_copy(xT[:, s:s + 128 * w], pxt)

            def ctx_mm_one(j):
                nc.tensor.matmul(ps_cx[:, 0:8], lhsT=x_bf_r[:, j],
                                 rhs=eb_r[:, j],
                                 start=(j == 0), stop=(j == N_TILES - 1))

            def ctx_fillers(c):
                return [(lambda j=16 * c + i: ctx_mm_one(j))
                        for i in range(N_TILE_EX)]

            def ctx_mms(c):
                for f in ctx_fillers(c):
                    f()

            def scores(j0, n):
                for i in range(n):
                    j = j0 + i
                    nc.tensor.matmul(ps_sc[:, j:j + 1],
                                     lhsT=ht[:, 128 * j:128 * (j + 1)],
                                     rhs=v_bf)

            def exp_c(c):
                nc.scalar.activation(eb_r[:, 16 * c:16 * c + 16, c],
                                     ps_sc[:, 16 * c:16 * c + 16],
                                     ACT.Exp, accum_out=e_all[:, c:c + 1])

            def h_mm(ph, base, lo, hi):
                nc.tensor.matmul(ph[:, lo:hi], lhsT=wc_bf,
                                 rhs=xT[:, base + lo:base + hi])

            for c in range(B_LOC - 1):
                if c >= 1:
                    # ACT queue: previous example's exp runs while PE does
                    # this example's transposes/h-matmuls
                    exp_c(c - 1)
                # ctx(c-1) matmuls ride inside this example's SECOND
                # transpose cluster (2 per transpose; their ldweights hide
                # under the transpose streams, and by then exp(c-1) is
                # done).
                fillers = ctx_fillers(c - 1) if c >= 1 else []
                for g in range(2):
                    base = 2048 * c + 1024 * g
                    j0 = 16 * c + 8 * g
                    ph = ps_h_pool.tile([128, 1024], f32, tag="ph")
                    fill = fillers if g == 1 else None
                    if c == 0:   # 4-tile chains for the ramp
                        transposes(j0, 4, None)
                        h_mm(ph, base, 0, 512)
                        nc.scalar.activation(ht[:, base:base + 512],
                                             ph[:, 0:512], ACT.Tanh,
                                             bias=b_sum)
                        transposes(j0 + 4, 4, None)
                        h_mm(ph, base, 512, 1024)
                        nc.scalar.activation(ht[:, base + 512:base + 1024],
                                             ph[:, 512:1024], ACT.Tanh,
                                             bias=b_sum)
                    else:
                        transposes(j0, 8, fill)
                        h_mm(ph, base, 0, 512)
                        h_mm(ph, base, 512, 1024)
                        nc.scalar.activation(ht[:, base:base + 1024], ph,
                                             ACT.Tanh, bias=b_sum)
                scores(16 * c, 8)
                scores(16 * c + 8, 8)

            # ---- last example: fully pipelined ever-finer tail ----
            # chunks of 4,4,4,2,2 tiles; tanh/scores/exp/ctx per chunk
            c = B_LOC - 1
            CH = [(112, 4), (116, 4), (120, 4), (124, 2), (126, 2)]
            e7q = [e_7a, e_7b,
                   big.tile([128, 1], f32, name="e_7c"),
                   big.tile([128, 1], f32, name="e_7d"),
                   big.tile([128, 1], f32, name="e_7e")]
            # ph tiles: chunks 0-1 share ph0; chunks 2-4 share ph1
            ph7 = [None, None]
            ph_of = [(0, 0), (0, 512), (1, 0), (1, 512), (1, 768)]

            def c7_chunk(q):
                j0, w = CH[q]
                t, lo = ph_of[q]
                if ph7[t] is None or (t, lo) in ((0, 0), (1, 0)):
                    ph7[t] = ps_h_pool.tile([128, 1024], f32, tag="ph",
                                            name="ph7")
                ph = ph7[t]
                transposes(j0, w)
                cols = 128 * w
                nc.tensor.matmul(ph[:, lo:lo + cols], lhsT=wc_bf,
                                 rhs=xT[:, 128 * j0:128 * j0 + cols])
                nc.scalar.activation(ht[:, 128 * j0:128 * j0 + cols],
                                     ph[:, lo:lo + cols], ACT.Tanh,
                                     bias=b_sum)

            def sc7(q):
                scores(CH[q][0], CH[q][1])

            def exp7(q):
                j0, w = CH[q]
                nc.scalar.activation(eb_r[:, j0:j0 + w, c],
                                     ps_sc[:, j0:j0 + w],
                                     ACT.Exp, accum_out=e7q[q])

            def ctx7(q):
                j0, w = CH[q]
                for i in range(w):
                    j = j0 + i
                    nc.tensor.matmul(ps_cx[:, 0:8], lhsT=x_bf_r[:, j],
                                     rhs=eb_r[:, j],
                                     start=(j == 0), stop=(j == N_TILES - 1))

            exp_c(c - 1)
            c7_chunk(0)
            c7_chunk(1)
            ctx_mms(c - 1)
            sc7(0); exp7(0)
            c7_chunk(2)
            sc7(1); exp7(1)
            ctx7(0)
            c7_chunk(3)
            sc7(2); exp7(2)
            ctx7(1)
            c7_chunk(4)
            sc7(3); exp7(3)
            ctx7(2)
            sc7(4); exp7(4)
            ctx7(3)
            ctx7(4)

            # ---- denominator + final transpose/scale ----
            s01 = big.tile([128, 1], f32)
            s23 = big.tile([128, 1], f32)
            s04 = big.tile([128, 1], f32)
            nc.vector.tensor_add(s01, e7q[0], e7q[1])
            nc.vector.tensor_add(s23, e7q[2], e7q[3])
            nc.vector.tensor_add(s04, s01, s23)
            nc.vector.tensor_add(e_all[:, c:c + 1], s04, e7q[4])
            nc.tensor.matmul(ps_cx[0:8, 448:449], lhsT=e_all, rhs=ones_col)

            nc.vector.tensor_copy(cxT_sb, ps_cx[:, 0:8])
            nc.tensor.transpose(ps_cx[0:8, 320:448], cxT_sb, ident_f)

            nc.vector.reciprocal(den_r, ps_cx[0:8, 448:449])
            nc.vector.tensor_scalar_mul(out_sb, ps_cx[0:8, 320:448], den_r)
            nc.sync.dma_start(out=out_d.ap(), in_=out_sb)

    nc.compile()
    return nc


def get_nc():
    global _nc
    if _nc is None:
        _nc = _build_nc()
    return _nc


def kernel(encoder_outputs, W1_w, W1_b, W2_w, W2_b, V_w, V_b):
    global LAST_RESULT
    from concourse.bass_utils import run_bass_kernel_spmd

    nc = get_nc()

    enc = np.ascontiguousarray(np.asarray(encoder_outputs, dtype=np.float32))
    rep = {
        "W1_w": np.ascontiguousarray(np.asarray(W1_w, np.float32)),
        "W1_b": np.ascontiguousarray(np.asarray(W1_b, np.float32).reshape(U, 1)),
        "W2_w": np.ascontiguousarray(np.asarray(W2_w, np.float32)),
        "W2_b": np.ascontiguousarray(np.asarray(W2_b, np.float32).reshape(U, 1)),
        "V_w": np.ascontiguousarray(np.asarray(V_w, np.float32).reshape(U, 1)),
    }
    in_maps = []
    for c in range(N_CORES):
        shard = enc[c * B_LOC:(c + 1) * B_LOC].reshape(B_LOC * T, D)
        in_maps.append({"encoder_outputs": np.ascontiguousarray(shard), **rep})

    trace = bool(int(os.environ.get("KERNEL_TRACE", "0")))
    LAST_RESULT = run_bass_kernel_spmd(
        nc, in_maps, core_ids=list(range(N_CORES)), trace=trace)
    out = np.concatenate(
        [LAST_RESULT.results[c]["out"] for c in range(N_CORES)], axis=0)
    return np.ascontiguousarray(out, dtype=np.float32)
